# revision 1
# baseline (speedup 1.0000x reference)
"""Multi-head attention kernel for Trainium2, 8 NeuronCores.

Problem (hardcoded shapes): B=4, S=2048, E=1024, H=16, DH=64.
  q/k/v = einsum('bse,hed->bhsd', x, W{q,k,v}) + b{q,k,v}
  attn  = softmax(q k^T / sqrt(DH)) v
  out   = concat_heads(attn) @ Wo^T + bo

Sharding: core c -> (batch b = c//2, head-half hh = c%2, i.e. heads
8*hh..8*hh+7).  Each core computes a [S, E] partial of its batch's output
(its 512 columns of concat against the matching 512 rows of Wo^T); the host
sums the two partials per batch and adds bo.

Per-core dataflow (everything transposed so the PE contraction dim lands on
SBUF partitions):
  xT   [e=128 x 8, s=2048]  via PE-transpose of x tiles       (fp32r)
  v    [t, d'=512]          x @ Wv for all 8 heads + bias      (bf16, with a
                            fused ones column per head -> softmax sums)
  qT/kT[j=128, s=2048]      per head-pair Wq^T @ xT + bias     (fp32r);
                            next pair's projection matmuls are interleaved
                            into the current pair's attention stream so the
                            scalar engine (softmax exp, the bottleneck) never
                            starves while the PE does projections
  scoresT [t=128, s=512]    kT-block^T as lhsT, qT as rhs      (PSUM fp32)
  expST               ACT Exp(scale=1/8) on 2-bank PSUM groups (bf16)
  attnT+sums [65, s]  vext as lhsT (M=65: 64 v cols + ones)    (PSUM fp32)
  normalize           DVE recip + gpsimd partition_broadcast + DVE mul
  concatT [f=128 x 4, s]    normalized attnT                   (bf16)
  out_partial [s, e]        concatT as lhsT, Wo^T as rhs       (fp32)
"""

import os
import sys

for _p in ("/opt/trn_rl_repo", "/root/.axon_site/_ro/trn_rl_repo"):
    if os.path.isdir(_p) and _p not in sys.path:
        sys.path.insert(0, _p)
        break

from contextlib import ExitStack

import numpy as np
import ml_dtypes

import concourse.bass as bass
import concourse.tile as tile
import concourse.mybir as mybir
from concourse import bacc, bass_utils

B, S, E, H, DH = 4, 2048, 1024, 16, 64
HPC = 8           # heads per core
JW = HPC * DH     # 512, per-core qkv width
N_CORES = 8
SB = S // 128     # 16 s-blocks / t-blocks
EB = E // 128     # 8 e-blocks
SC = S // 512     # 4 s-chunks
F32 = mybir.dt.float32
F32R = mybir.dt.float32r
BF16 = mybir.dt.bfloat16
Exp = mybir.ActivationFunctionType.Exp
MULT = mybir.AluOpType.mult
ADD = mybir.AluOpType.add


def _emit(tc, aps, ctx):
    nc = tc.nc
    x_d, wq_d, wk_d, wv_d, wo_d, bqt_d, bkt_d, bv_d, id_d, out_d = aps

    def pool(**kw):
        return ctx.enter_context(tc.tile_pool(**kw))

    const = pool(name="const", bufs=1)
    xs = pool(name="xs", bufs=3)
    xTp = pool(name="xT", bufs=1)
    vxp = pool(name="vext", bufs=1)
    wqk = pool(name="wqk", bufs=2)
    qkp = pool(name="qk", bufs=2)
    exp_p = pool(name="expS", bufs=3)
    ccp = pool(name="concatT", bufs=1)
    nrm = pool(name="nrm", bufs=2)
    outp = pool(name="outs", bufs=3)
    ps_sm = pool(name="ps_sm", bufs=2, space="PSUM")
    ps_sc = pool(name="ps_sc", bufs=2, space="PSUM")
    ps_ac = pool(name="ps_ac", bufs=2, space="PSUM")

    # ---- constants / weights (x tiles are DMA'd first in emit_ab; keep the
    # bulky weight loads from queuing ahead of them) ----
    ident = const.tile([128, 128], F32R)
    nc.sync.dma_start(ident[:], id_d[:])
    bq_sb = const.tile([128, 4], F32)
    nc.sync.dma_start(bq_sb[:], bqt_d[:])
    bk_sb = const.tile([128, 4], F32)
    nc.sync.dma_start(bk_sb[:], bkt_d[:])
    bv1 = const.tile([1, JW], F32)
    nc.sync.dma_start(bv1[:], bv_d[:])
    bvb = const.tile([128, JW], F32)
    nc.gpsimd.partition_broadcast(bvb[:], bv1[:])
    wv_sb = const.tile([128, EB, JW], F32R)
    wo_sb = const.tile([128, 4, E], BF16)

    def load_wv():
        nc.sync.dma_start(wv_sb[:], wv_d.rearrange("(eb p) j -> p eb j", p=128))

    def load_wo():
        nc.sync.dma_start(wo_sb[:], wo_d.rearrange("(fb p) e -> p fb e", p=128))

    xT = xTp.tile([128, EB, S], F32R)
    vext = vxp.tile([128, SB, HPC, DH + 1], BF16)

    def dma_x(sb):
        x_t = xs.tile([128, E], F32R, tag="x_t", name=f"x_t_{sb}")
        nc.sync.dma_start(x_t[:], x_d[sb * 128:(sb + 1) * 128, :])
        return x_t

    def emit_tr(sb, x_t):
        """Transpose x s-block sb into xT."""
        for half in range(2):
            pt = ps_sm.tile([128, 512], F32R, tag="ps_sm", name=f"pt{sb}_{half}")
            for q in range(4):
                eb = half * 4 + q
                nc.tensor.transpose(pt[:, q * 128:(q + 1) * 128],
                                    x_t[:, eb * 128:(eb + 1) * 128], ident[:])
            nc.vector.tensor_copy(
                xT[:, half * 4:(half + 1) * 4, sb * 128:(sb + 1) * 128],
                pt[:].rearrange("p (e s) -> p e s", e=4))

    def emit_v(sb):
        """Project v (all 8 heads) for t-block sb into vext."""
        pv = ps_sm.tile([128, 512], F32, tag="ps_sm", name=f"pv{sb}")
        for eb in range(EB):
            nc.tensor.matmul(pv[:], xT[:, eb, sb * 128:(sb + 1) * 128],
                             wv_sb[:, eb, :],
                             start=(eb == 0), stop=(eb == EB - 1))
        nc.vector.tensor_tensor(
            vext[:, sb, :, 0:DH],
            pv[:].rearrange("p (h d) -> p h d", h=HPC),
            bvb[:].rearrange("p (h d) -> p h d", h=HPC), ADD)

    # ---- per-pair projection helpers ----
    def load_pair_weights(p):
        wq_t = wqk.tile([128, EB, 128], F32R, tag="wq")
        nc.sync.dma_start(
            wq_t[:], wq_d.rearrange("(eb pp) j -> pp eb j", pp=128)[
                :, :, p * 128:(p + 1) * 128])
        wk_t = wqk.tile([128, EB, 128], F32R, tag="wk")
        nc.sync.dma_start(
            wk_t[:], wk_d.rearrange("(eb pp) j -> pp eb j", pp=128)[
                :, :, p * 128:(p + 1) * 128])
        qT = qkp.tile([128, S], F32R, tag="qT")
        kT = qkp.tile([128, S], F32R, tag="kT")
        return wq_t, wk_t, qT, kT

    def proj_chunks(p, wq_t, wk_t, qT, kT):
        """One closure per (s-chunk, q|k): 8 matmuls + bias copy."""
        chunks = []
        for sc in range(SC):
            for w_t, dst, b_sb in ((wq_t, qT, bq_sb), (wk_t, kT, bk_sb)):
                def emit(sc=sc, w_t=w_t, dst=dst, b_sb=b_sb):
                    pq = ps_sm.tile([128, 512], F32, tag="ps_sm")
                    for eb in range(EB):
                        nc.tensor.matmul(pq[:], w_t[:, eb, :],
                                         xT[:, eb, sc * 512:(sc + 1) * 512],
                                         start=(eb == 0), stop=(eb == EB - 1))
                    nc.vector.tensor_scalar_add(
                        dst[:, sc * 512:(sc + 1) * 512], pq[:], b_sb[:, p:p + 1])
                chunks.append(emit)
        return chunks

    concatT = ccp.tile([128, 4, S], BF16)
    pair_qk = {}

    def normalize(acc, bp, p, sc):
        """attnT[d, s] / sums[s] -> concatT slice."""
        r_t = nrm.tile([1, 512], F32, tag="r")
        nc.vector.reciprocal(r_t[:], acc[64:65, :])
        rb_t = nrm.tile([64, 512], F32, tag="rb")
        nc.gpsimd.partition_broadcast(rb_t[:], r_t[:])
        st = nrm.tile([64, 512], BF16, tag="st")
        nc.vector.tensor_tensor(st[:], acc[0:64, :], rb_t[:], MULT)
        nc.sync.dma_start(
            concatT[bp:bp + 64, p, sc * 512:(sc + 1) * 512], st[:])

    def outproj_chunk(sb, ec, alt=False):
        def emit():
            pp = ps_sc if alt else ps_sm
            po = pp.tile([128, 512], F32, tag="sc" if alt else "ps_sm",
                         name=f"po_{sb}_{ec}")
            for fb in range(4):
                nc.tensor.matmul(po[:],
                                 concatT[:, fb, sb * 128:(sb + 1) * 128],
                                 wo_sb[:, fb, ec * 512:(ec + 1) * 512],
                                 start=(fb == 0), stop=(fb == 3))
            ot = outp.tile([128, 512], F32, tag="ot", name=f"ot_{sb}_{ec}")
            nc.vector.tensor_copy(ot[:], po[:])
            nc.sync.dma_start(
                out_d[sb * 128:(sb + 1) * 128, ec * 512:(ec + 1) * 512], ot[:])
        return emit

    out_chunks = []   # filled as pair-3 s-chunks complete

    # One global software-pipelined stream over every attention group:
    # attnT for group i is emitted after the scores+exp of group i+1, across
    # (pair, head, s-chunk) boundaries, so the PE never drains waiting on the
    # scalar engine at iteration boundaries.  Next pair's projections are
    # injected into the PE stream at a fixed cadence, and output-projection
    # chunks fill the PE slack during pair 3 (which has no next pair).
    INJ_EVERY = 8
    OUT_EVERY = 4

    def attn_stream():
        iters = [(p, hl, sc) for p in range(3) for hl in range(2)
                 for sc in range(SC)]
        # pair 3: s-chunk-major so outproj chunks unlock mid-pair
        iters += [(3, hl, sc) for sc in range(SC) for hl in range(2)]
        accs = {}
        pend = [None]     # (ex tile, iter key, g, acc, h)
        inj = []          # pending projection chunks for the upcoming pair
        cur_pair = -1
        gctr = 0

        def flush_pend():
            if pend[0] is None:
                return
            pex, key, pg, acc, h = pend[0]
            for t2 in range(2):
                tb = pg * 2 + t2
                nc.tensor.matmul(acc[:], vext[:, tb, h, :],
                                 pex[:, t2 * 512:(t2 + 1) * 512],
                                 start=(pg == 0 and t2 == 0),
                                 stop=(pg == 7 and t2 == 1))
            if pg == 7:
                p, hl, sc = key
                normalize(acc, hl * 64, p, sc)
                del accs[key]
                if p == 3 and hl == 1:
                    for sb in range(4 * sc, 4 * sc + 4):
                        for ec in range(2):
                            out_chunks.append(outproj_chunk(sb, ec))
            pend[0] = None

        for key in iters:
            p, hl, sc = key
            if p != cur_pair:
                cur_pair = p
                if p == 2:
                    load_wo()
                if p + 1 < 4:
                    nstate = load_pair_weights(p + 1)
                    inj.extend(proj_chunks(p + 1, *nstate))
                    pair_qk[p + 1] = (nstate[2], nstate[3])
            qT, kT = pair_qk[p]
            h = p * 2 + hl
            bp = hl * 64
            qs = qT[bp:bp + 64, sc * 512:(sc + 1) * 512]
            accs[key] = ps_ac.tile([65, 512], F32, tag="acc",
                                   name=f"acc_{p}_{hl}_{sc}")
            for g in range(8):
                scp = ps_sc.tile([128, 1024], F32, tag="sc",
                                 name=f"scp_{p}_{hl}_{sc}_{g}")
                for t2 in range(2):
                    tb = g * 2 + t2
                    nc.tensor.matmul(scp[:, t2 * 512:(t2 + 1) * 512],
                                     kT[bp:bp + 64, tb * 128:(tb + 1) * 128],
                                     qs, start=True, stop=True)
                ex = exp_p.tile([128, 1024], BF16, tag="ex",
                                name=f"ex_{p}_{hl}_{sc}_{g}")
                nc.scalar.activation(ex[:], scp[:], Exp, scale=0.125)
                flush_pend()
                pend[0] = (ex, key, g, accs[key], h)
                gctr += 1
                if inj and gctr % INJ_EVERY == 0:
                    inj.pop(0)()
                elif out_chunks and gctr % OUT_EVERY == 0:
                    out_chunks.pop(0)()
                yield
        flush_pend()
        for ch in inj:  # leftovers, if any
            ch()

    # ---- prefix: transposes + v, with pair-0 projections and the first
    # attention groups interleaved so the scalar engine starts early ----
    nc.gpsimd.memset(vext[:, :, :, DH:DH + 1], 1.0)
    x0 = dma_x(0)
    load_wv()
    state = load_pair_weights(0)
    p0_chunks = proj_chunks(0, *state)  # [sc0q, sc0k, sc1q, sc1k, ...]
    pair_qk[0] = (state[2], state[3])
    gen = attn_stream()

    def pump(n):
        for _ in range(n):
            next(gen, None)

    x_tiles = {0: x0, 1: dma_x(1)}
    for sb in range(SB):
        if sb + 2 < SB:
            x_tiles[sb + 2] = dma_x(sb + 2)  # 2-deep DMA prefetch
        emit_tr(sb, x_tiles.pop(sb))
        if sb >= 2:
            emit_v(sb - 2)           # lag v so the wv DMA has landed
        if sb % 4 == 3:
            ch = sb // 4
            p0_chunks[2 * ch]()      # q chunk sc=ch
            p0_chunks[2 * ch + 1]()  # k chunk sc=ch
        if sb >= 4 and sb % 2 == 0:
            pump(1)                  # groups 0..5 at sb 4,6,8,10,12,14
    emit_v(SB - 2)
    emit_v(SB - 1)
    for _ in gen:
        pass

    # ---- remaining output-projection chunks (alternate PSUM pools so four
    # chunks can be in flight; the scores pool is idle by now) ----
    pairs = [(sb, ec) for sc in range(SC) for sb in range(4 * sc, 4 * sc + 4)
             for ec in range(2)]
    rem = pairs[-len(out_chunks):] if out_chunks else []
    for i, (sb, ec) in enumerate(rem):
        outproj_chunk(sb, ec, alt=(i % 2 == 1))()


_CACHE = {}


def _build():
    nc = bacc.Bacc("TRN2", target_bir_lowering=False, debug=False,
                   num_devices=N_CORES)
    x_d = nc.dram_tensor("x", [S, E], F32R, kind="ExternalInput").ap()
    wq_d = nc.dram_tensor("wq", [E, JW], F32R, kind="ExternalInput").ap()
    wk_d = nc.dram_tensor("wk", [E, JW], F32R, kind="ExternalInput").ap()
    wv_d = nc.dram_tensor("wv", [E, JW], F32R, kind="ExternalInput").ap()
    wo_d = nc.dram_tensor("wo", [JW, E], BF16, kind="ExternalInput").ap()
    bqt_d = nc.dram_tensor("bqt", [128, 4], F32, kind="ExternalInput").ap()
    bkt_d = nc.dram_tensor("bkt", [128, 4], F32, kind="ExternalInput").ap()
    bv_d = nc.dram_tensor("bv", [1, JW], F32, kind="ExternalInput").ap()
    id_d = nc.dram_tensor("ident", [128, 128], F32R, kind="ExternalInput").ap()
    out_d = nc.dram_tensor("out", [S, E], F32, kind="ExternalOutput").ap()
    aps = (x_d, wq_d, wk_d, wv_d, wo_d, bqt_d, bkt_d, bv_d, id_d, out_d)
    with tile.TileContext(nc) as tc:
        with ExitStack() as ctx:
            _emit(tc, aps, ctx)
    nc.compile()
    return nc


def kernel(x, Wq, bq, Wk, bk, Wv, bv, Wo, bo):
    x = np.asarray(x, dtype=np.float32)
    Wq = np.asarray(Wq, dtype=np.float32)
    bq = np.asarray(bq, dtype=np.float32)
    Wk = np.asarray(Wk, dtype=np.float32)
    bk = np.asarray(bk, dtype=np.float32)
    Wv = np.asarray(Wv, dtype=np.float32)
    bv = np.asarray(bv, dtype=np.float32)
    Wo = np.asarray(Wo, dtype=np.float32)
    bo = np.asarray(bo, dtype=np.float32)

    if "nc" not in _CACHE:
        _CACHE["nc"] = _build()
    nc = _CACHE["nc"]

    WoT = np.ascontiguousarray(Wo.T)  # [f, e]
    in_maps = []
    for c in range(N_CORES):
        b, hh = c // 2, c % 2
        hs = slice(hh * HPC, (hh + 1) * HPC)
        in_maps.append({
            "x": np.ascontiguousarray(x[b]),
            "wq": np.ascontiguousarray(
                Wq[hs].transpose(1, 0, 2).reshape(E, JW)),
            "wk": np.ascontiguousarray(
                Wk[hs].transpose(1, 0, 2).reshape(E, JW)),
            "wv": np.ascontiguousarray(
                Wv[hs].transpose(1, 0, 2).reshape(E, JW)),
            "wo": np.ascontiguousarray(
                WoT[hh * JW:(hh + 1) * JW]).astype(ml_dtypes.bfloat16),
            "bqt": np.ascontiguousarray(bq[hs].reshape(4, 128).T),
            "bkt": np.ascontiguousarray(bk[hs].reshape(4, 128).T),
            "bv": bv[hs].reshape(1, JW),
            "ident": np.eye(128, dtype=np.float32),
        })

    res = bass_utils.run_bass_kernel_spmd(nc, in_maps,
                                          core_ids=list(range(N_CORES)))
    out = np.empty((B, S, E), dtype=np.float32)
    for b in range(B):
        out[b] = res.results[2 * b]["out"] + res.results[2 * b + 1]["out"]
        out[b] += bo[None, :]
    return out

